# revision 2
# baseline (speedup 1.0000x reference)
"""TRN2 Bass kernel for nn_BSLinear_71159018160311.

Computes  out = input @ W.T  with
  W = U @ diag(weight^2 * mask) @ Vh + U_additional @ Vh_additional

Sharding: data-parallel over the B*S=16384 token dim across 8 NeuronCores
(2048 tokens/core), no collectives. Each core runs the factorized form as
two fused matmul phases in float32r (full-rate fp32 streaming on the PE):

  phase 1: t = V_eff @ x_c.T   kept entirely in SBUF (r-major, [RP, 2048])
           k-blocked PSUM accumulation (4 k-tiles/block) + SBUF adds
  phase 2: yT_c = U_eff @ t    (ut streamed once, 512-col chunks; output
           dout-major, host transposes back)

V_eff = [Vh; Vh_additional(pad)]  (rows), U_eff = [U*s, U_additional(pad)]
(cols), s = weight^2*mask folded on host. When U_additional/Vh_additional
are all-zero (they are for this problem instance), the padded tail is
dropped (NR=8 -> RP=1024), saving 11% of the matmul work; otherwise the
NR=9 (RP=1152) program handles the full module.

HBM traffic per core is at the floor: x 32MB + vt 16MB + ut 16MB + y 32MB
(the [RP,2048] intermediate never touches DRAM).
"""

import functools

import numpy as np

B, S, D_IN, D_OUT, R, A = 4, 4096, 4096, 4096, 1024, 64
N_CORES = 8
T = B * S
TC = T // N_CORES  # 2048
KT = D_IN // 128  # 32
KB = 4
NB = KT // KB
NN = TC // 512  # 4
ND = D_OUT // 512  # 8


@functools.lru_cache(maxsize=2)
def _build(NR):
    import concourse.bacc as bacc
    import concourse.mybir as mybir
    import concourse.tile as tile

    RP = NR * 128
    f32r = mybir.dt.float32r
    f32 = mybir.dt.float32
    add = mybir.AluOpType.add

    nc = bacc.Bacc(trn_type="TRN2")
    with tile.TileContext(nc) as tc:
        with tc.tile_pool(name="dram", bufs=1, space="DRAM") as dram:
            xT = dram.tile([D_IN, TC], f32r, kind="ExternalInput", name="xT")
            vt = dram.tile([D_IN, RP], f32r, kind="ExternalInput", name="vt")
            ut = dram.tile([RP, D_OUT], f32r, kind="ExternalInput", name="ut")
            yT = dram.tile([D_OUT, TC], f32, kind="ExternalOutput", name="yT")

            with (
                tc.tile_pool(name="tsb", bufs=NR) as tpool,
                tc.tile_pool(name="ut0", bufs=1) as u0pool,
            ):
                t_sb = [tpool.tile([128, TC], f32r, name="tsb") for _ in range(NR)]
                # first ut chunk: loads during phase 1 (own address space);
                # DMA emitted after block-0 loads so it doesn't delay startup
                ut0 = u0pool.tile([128, NR, 512], f32r)

                # ---- phase 1 ----
                with (
                    tc.tile_pool(name="xk", bufs=2 * KB) as xpool,
                    tc.tile_pool(name="vk", bufs=2 * KB) as vpool,
                    tc.tile_pool(name="ps1", bufs=2, space="PSUM") as pspool,
                ):
                    for kb in range(NB):
                        xts, vts = [], []
                        for j in range(KB):
                            k = kb * KB + j
                            xt_t = xpool.tile([128, TC], f32r, name="xk")
                            nc.sync.dma_start(xt_t[:], xT[k * 128:(k + 1) * 128, :])
                            vt_t = vpool.tile([128, RP], f32r, name="vk")
                            nc.sync.dma_start(vt_t[:], vt[k * 128:(k + 1) * 128, :])
                            xts.append(xt_t)
                            vts.append(vt_t)
                        if kb == 0:
                            nc.sync.dma_start(
                                ut0[:],
                                ut[:, 0:512].rearrange("(ko p) f -> p ko f", p=128),
                            )
                        for r in range(NR):
                            psum = pspool.tile([128, NN, 512], f32, name="ps1")
                            for j in range(KB):
                                for n in range(NN):
                                    nc.tensor.matmul(
                                        psum[:, n, :],
                                        lhsT=vts[j][:, r * 128:(r + 1) * 128],
                                        rhs=xts[j][:, n * 512:(n + 1) * 512],
                                        start=(j == 0),
                                        stop=(j == KB - 1),
                                    )
                            for n in range(NN):
                                dst = t_sb[r][:, n * 512:(n + 1) * 512]
                                if kb == 0:
                                    nc.any.tensor_copy(dst, psum[:, n, :])
                                else:
                                    nc.any.tensor_tensor(
                                        dst, dst, psum[:, n, :], add
                                    )

                # ---- phase 2 (ut stationary, t moving; output dout-major) ----
                with (
                    tc.tile_pool(name="utd", bufs=2) as upool,
                    tc.tile_pool(name="ysb", bufs=8) as ypool,
                    tc.tile_pool(name="ps2", bufs=2, space="PSUM") as ps2pool,
                ):
                    for d in range(ND):
                        if d == 0:
                            ut_t = ut0
                        else:
                            ut_t = upool.tile([128, NR, 512], f32r, name="utd")
                            nc.sync.dma_start(
                                ut_t[:],
                                ut[:, d * 512:(d + 1) * 512].rearrange(
                                    "(ko p) f -> p ko f", p=128
                                ),
                            )
                        for dd in range(4):  # 128-wide dout sub-blocks
                            psum = ps2pool.tile([128, NN, 512], f32, name="ps2")
                            for r in range(NR):
                                for n in range(NN):
                                    nc.tensor.matmul(
                                        psum[:, n, :],
                                        lhsT=ut_t[:, r, dd * 128:(dd + 1) * 128],
                                        rhs=t_sb[r][:, n * 512:(n + 1) * 512],
                                        start=(r == 0),
                                        stop=(r == NR - 1),
                                    )
                            row = d * 512 + dd * 128
                            for n in range(NN):
                                ysb = ypool.tile([128, 512], f32, name="ysb")
                                nc.any.tensor_copy(ysb[:], psum[:, n, :])
                                nc.sync.dma_start(
                                    yT[row : row + 128, n * 512:(n + 1) * 512],
                                    ysb[:],
                                )
    nc.compile()
    return nc, xT.name, vt.name, ut.name, yT.name


def _prep_maps(input, weight, U, Vh, U_additional, Vh_additional, mask, names, NR):
    xT_name, vt_name, ut_name = names
    RP = NR * 128
    s = weight * weight * mask
    U_eff = np.zeros((D_OUT, RP), np.float32)
    U_eff[:, :R] = U * s[None, :]
    V_eff = np.zeros((RP, D_IN), np.float32)
    V_eff[:R] = Vh
    if NR > R // 128:
        U_eff[:, R : R + A] = U_additional
        V_eff[R : R + A] = Vh_additional
    vt = np.ascontiguousarray(V_eff.T)
    ut = np.ascontiguousarray(U_eff.T)
    x2 = np.asarray(input, dtype=np.float32).reshape(T, D_IN)
    in_maps = []
    for c in range(N_CORES):
        xTc = np.ascontiguousarray(x2[c * TC : (c + 1) * TC].T)
        in_maps.append({xT_name: xTc, vt_name: vt, ut_name: ut})
    return in_maps


def _gather(results, yT_name):
    out = np.empty((T, D_OUT), np.float32)
    for c in range(N_CORES):
        out[c * TC : (c + 1) * TC] = results[c][yT_name].T
    return out.reshape(B, S, D_OUT)


def _pick_nr(U_additional, Vh_additional):
    if not np.asarray(U_additional).any() or not np.asarray(Vh_additional).any():
        return R // 128  # additional term contributes nothing
    return (R + A + 127) // 128


def kernel(input, weight, U, Vh, U_additional, Vh_additional, mask, **_kw):
    from concourse.bass_utils import run_bass_kernel_spmd

    NR = _pick_nr(U_additional, Vh_additional)
    nc, xT_name, vt_name, ut_name, yT_name = _build(NR)
    in_maps = _prep_maps(
        input, weight, U, Vh, U_additional, Vh_additional, mask,
        (xT_name, vt_name, ut_name), NR,
    )
    res = run_bass_kernel_spmd(nc, in_maps, core_ids=list(range(N_CORES)))
    return _gather(res.results, yT_name)


# revision 3
# speedup vs baseline: 1.0513x; 1.0513x over previous
"""TRN2 Bass kernel for nn_BSLinear_71159018160311.

Computes  out = input @ W.T  with
  W = U @ diag(weight^2 * mask) @ Vh + U_additional @ Vh_additional

Sharding: data-parallel over the B*S=16384 token dim across 8 NeuronCores
(2048 tokens/core), no collectives. Each core runs the factorized form as
two fused matmul phases in float32r (full-rate fp32 streaming on the PE):

  phase 1: t = V_eff @ x_c.T   kept entirely in SBUF (r-major, [RP, 2048])
           k-blocked PSUM accumulation (4 k-tiles/block) + SBUF adds
  phase 2: yT_c = U_eff @ t    (ut streamed once, 512-col chunks; output
           dout-major, host transposes back)

V_eff = [Vh; Vh_additional(pad)]  (rows), U_eff = [U*s, U_additional(pad)]
(cols), s = weight^2*mask folded on host. When U_additional/Vh_additional
are all-zero (they are for this problem instance), the padded tail is
dropped (NR=8 -> RP=1024), saving 11% of the matmul work; otherwise the
NR=9 (RP=1152) program handles the full module.

HBM traffic per core is at the floor: x 32MB + vt 16MB + ut 16MB + y 32MB
(the [RP,2048] intermediate never touches DRAM).
"""

import functools

import numpy as np

B, S, D_IN, D_OUT, R, A = 4, 4096, 4096, 4096, 1024, 64
N_CORES = 8
T = B * S
TC = T // N_CORES  # 2048
KT = D_IN // 128  # 32
KB = 4
NB = KT // KB
NN = TC // 512  # 4
ND = D_OUT // 512  # 8


@functools.lru_cache(maxsize=2)
def _build(NR):
    import concourse.bacc as bacc
    import concourse.mybir as mybir
    import concourse.tile as tile

    RP = NR * 128
    f32r = mybir.dt.float32r
    f32 = mybir.dt.float32
    add = mybir.AluOpType.add

    nc = bacc.Bacc(trn_type="TRN2")
    with tile.TileContext(nc) as tc:
        with tc.tile_pool(name="dram", bufs=1, space="DRAM") as dram:
            xT = dram.tile([D_IN, TC], f32r, kind="ExternalInput", name="xT")
            vt = dram.tile([D_IN, RP], f32r, kind="ExternalInput", name="vt")
            ut = dram.tile([RP, D_OUT], f32r, kind="ExternalInput", name="ut")
            yT = dram.tile([D_OUT, TC], f32, kind="ExternalOutput", name="yT")

            with (
                tc.tile_pool(name="tsb", bufs=NR) as tpool,
                tc.tile_pool(name="ut0", bufs=1) as u0pool,
            ):
                t_sb = [tpool.tile([128, TC], f32r, name="tsb") for _ in range(NR)]
                # first ut chunk: loads during phase 1 (own address space);
                # DMA emitted after block-0 loads so it doesn't delay startup
                ut0 = u0pool.tile([128, NR, 512], f32r)

                # ---- phase 1 ----
                with (
                    tc.tile_pool(name="xk", bufs=2 * KB) as xpool,
                    tc.tile_pool(name="vk", bufs=2 * KB) as vpool,
                    tc.tile_pool(name="ps1", bufs=2, space="PSUM") as pspool,
                ):
                    for kb in range(NB):
                        xts, vts = [], []
                        for j in range(KB):
                            k = kb * KB + j
                            xt_t = xpool.tile([128, TC], f32r, name="xk")
                            nc.sync.dma_start(xt_t[:], xT[k * 128:(k + 1) * 128, :])
                            vt_t = vpool.tile([128, RP], f32r, name="vk")
                            nc.sync.dma_start(vt_t[:], vt[k * 128:(k + 1) * 128, :])
                            xts.append(xt_t)
                            vts.append(vt_t)
                        if kb == 0:
                            nc.sync.dma_start(
                                ut0[:],
                                ut[:, 0:512].rearrange("(ko p) f -> p ko f", p=128),
                            )
                        for r in range(NR):
                            psum = pspool.tile([128, NN, 512], f32, name="ps1")
                            for j in range(KB):
                                for n in range(NN):
                                    nc.tensor.matmul(
                                        psum[:, n, :],
                                        lhsT=vts[j][:, r * 128:(r + 1) * 128],
                                        rhs=xts[j][:, n * 512:(n + 1) * 512],
                                        start=(j == 0),
                                        stop=(j == KB - 1),
                                    )
                            for n in range(NN):
                                dst = t_sb[r][:, n * 512:(n + 1) * 512]
                                if kb == 0:
                                    nc.any.tensor_copy(dst, psum[:, n, :])
                                else:
                                    nc.any.tensor_tensor(
                                        dst, dst, psum[:, n, :], add
                                    )

                # ---- phase 2 (ut stationary, t moving; output dout-major) ----
                with (
                    tc.tile_pool(name="utd", bufs=2) as upool,
                    tc.tile_pool(name="ysb", bufs=8) as ypool,
                    tc.tile_pool(name="ps2", bufs=2, space="PSUM") as ps2pool,
                ):
                    for d in range(ND):
                        if d == 0:
                            ut_t = ut0
                        else:
                            ut_t = upool.tile([128, NR, 512], f32r, name="utd")
                            nc.sync.dma_start(
                                ut_t[:],
                                ut[:, d * 512:(d + 1) * 512].rearrange(
                                    "(ko p) f -> p ko f", p=128
                                ),
                            )
                        for dd in range(4):  # 128-wide dout sub-blocks
                            psum = ps2pool.tile([128, NN, 512], f32, name="ps2")
                            for r in range(NR):
                                for n in range(NN):
                                    nc.tensor.matmul(
                                        psum[:, n, :],
                                        lhsT=ut_t[:, r, dd * 128:(dd + 1) * 128],
                                        rhs=t_sb[r][:, n * 512:(n + 1) * 512],
                                        start=(r == 0),
                                        stop=(r == NR - 1),
                                    )
                            row = d * 512 + dd * 128
                            for n in range(NN):
                                ysb = ypool.tile([128, 512], f32, name="ysb")
                                nc.any.tensor_copy(ysb[:], psum[:, n, :])
                                nc.sync.dma_start(
                                    yT[row : row + 128, n * 512:(n + 1) * 512],
                                    ysb[:],
                                )
    nc.compile()
    return nc, xT.name, vt.name, ut.name, yT.name


def _prep_maps(input, weight, U, Vh, U_additional, Vh_additional, mask, names, NR):
    xT_name, vt_name, ut_name = names
    RP = NR * 128
    s = weight * weight * mask
    U_eff = np.zeros((D_OUT, RP), np.float32)
    U_eff[:, :R] = U * s[None, :]
    V_eff = np.zeros((RP, D_IN), np.float32)
    V_eff[:R] = Vh
    if NR > R // 128:
        U_eff[:, R : R + A] = U_additional
        V_eff[R : R + A] = Vh_additional
    vt = np.ascontiguousarray(V_eff.T)
    ut = np.ascontiguousarray(U_eff.T)
    x2 = np.asarray(input, dtype=np.float32).reshape(T, D_IN)
    in_maps = []
    for c in range(N_CORES):
        xTc = np.ascontiguousarray(x2[c * TC : (c + 1) * TC].T)
        in_maps.append({xT_name: xTc, vt_name: vt, ut_name: ut})
    return in_maps


def _gather(results, yT_name):
    out = np.empty((T, D_OUT), np.float32)
    for c in range(N_CORES):
        out[c * TC : (c + 1) * TC] = results[c][yT_name].T
    return out.reshape(B, S, D_OUT)


def _pick_nr(U_additional, Vh_additional):
    if not np.asarray(U_additional).any() or not np.asarray(Vh_additional).any():
        return R // 128  # additional term contributes nothing
    return (R + A + 127) // 128


def kernel(input, weight, U, Vh, U_additional, Vh_additional, mask, **_kw):
    from concourse.bass_utils import run_bass_kernel_spmd

    input = np.asarray(input, dtype=np.float32)
    weight = np.asarray(weight, dtype=np.float32)
    U = np.asarray(U, dtype=np.float32)
    Vh = np.asarray(Vh, dtype=np.float32)
    U_additional = np.asarray(U_additional, dtype=np.float32)
    Vh_additional = np.asarray(Vh_additional, dtype=np.float32)
    mask = np.asarray(mask, dtype=np.float32)

    NR = _pick_nr(U_additional, Vh_additional)
    nc, xT_name, vt_name, ut_name, yT_name = _build(NR)
    in_maps = _prep_maps(
        input, weight, U, Vh, U_additional, Vh_additional, mask,
        (xT_name, vt_name, ut_name), NR,
    )
    res = run_bass_kernel_spmd(nc, in_maps, core_ids=list(range(N_CORES)))
    return _gather(res.results, yT_name)
